# revision 11
# baseline (speedup 1.0000x reference)
"""Trainium2 Bass kernel for nn_GCN_12979391169341 (GNN message passing).

Reference computation (N=2048 nodes, B=16, C_IN=32, C_OUT=64, E=16, K=3):
    A1 = A[1]
    conv_E_l = A1 @ conv_E
    scores = relu(conv_E_l @ conv_E.T)
    supports S = softmax(scores, axis=1)
    S_set = [I, S, 2*S@S - I]           (Chebyshev)
    x_g[b,n,k,c] = sum_m S_k[n,m] x[b,m,c]
    out[b,n,o]   = sum_{k,i} x_g[b,n,k,i] * weight[n,k,i,o] + b

Sharding: node-parallel over 8 cores (256 nodes each). Each core computes
its 256 rows of S locally (softmax rows are independent), computes its rows
of x_g1 = S@x, all-gathers x_g1 (4MB) so x_g2 = 2*S@(S@x) - x avoids ever
materializing S^2, then applies the per-node weights.

Key layout trick: S is computed in natural [n, m] layout (softmax along the
free axis), then PE-transposed to [m, n] so it can serve as the stationary
operand of every downstream matmul.
"""

import numpy as np
import sys

sys.path.insert(0, "/opt/trn_rl_repo")

import concourse.bass as bass
import concourse.mybir as mybir
import concourse.tile as tile
from concourse.bass_utils import run_bass_kernel_spmd

N = 2048      # nodes
B = 16        # batch
CI = 32       # in channels
CO = 64       # out channels
E = 16        # conv_E dim
KCH = 3       # Chebyshev order
NCORE = 8
NL = N // NCORE          # 256 local nodes
BC = B * CI              # 512
KI = KCH * CI            # 96
DT = mybir.dt.float32

_CACHE = {}
_RUN_KWARGS = {}
_LAST_RESULT = [None]


def _build():
    nc = bass.Bass(num_devices=NCORE)

    # ---- I/O ----
    a1t = nc.dram_tensor("a1t", [N, NL], DT, kind="ExternalInput")      # A1[nc,:].T
    et = nc.dram_tensor("et", [E, N], DT, kind="ExternalInput")         # conv_E.T
    e_nat = nc.dram_tensor("e_nat", [N, E], DT, kind="ExternalInput")   # conv_E
    xfull = nc.dram_tensor("xfull", [N, BC], DT, kind="ExternalInput")  # X[m, b*CI+i]
    xct = nc.dram_tensor("xct", [BC, NL], DT, kind="ExternalInput")     # X[nc,:].T
    xgt0 = nc.dram_tensor("xgt0", [CI, B * NL], DT, kind="ExternalInput")  # [i, b*NL+n]
    wt = nc.dram_tensor("wt", [KI, NL * CO], DT, kind="ExternalInput")  # [ki, n*CO+o]
    ident = nc.dram_tensor("ident", [128, 128], DT, kind="ExternalInput")
    outd = nc.dram_tensor("out", [B, NL, CO], DT, kind="ExternalOutput")

    # internal DRAM for the collective
    xg1_own = nc.dram_tensor("xg1_own", [NL, BC], DT)
    xg1_all = nc.dram_tensor("xg1_all", [N, BC], DT)

    MCH = N // 128  # 16 m-chunks

    with tile.TileContext(nc) as tc:
        with (
            tc.tile_pool(name="const", bufs=1) as cpool,
            tc.tile_pool(name="stream", bufs=3) as spool,
            tc.tile_pool(name="wstream", bufs=2) as wpool,
            tc.tile_pool(name="psum", bufs=2, space="PSUM") as pp,
        ):
            # ---- resident loads ----
            et_sb = cpool.tile([E, N], DT)
            nc.sync.dma_start(et_sb[:], et[:])
            e_sb = cpool.tile([128, MCH, E], DT)
            nc.sync.dma_start(e_sb[:], e_nat.rearrange("(c p) e -> p c e", p=128))
            xf_sb = cpool.tile([128, MCH, BC], DT)
            nc.sync.dma_start(xf_sb[:], xfull.rearrange("(c p) n -> p c n", p=128))
            xct_sb = cpool.tile([128, 4, NL], DT)
            nc.sync.dma_start(xct_sb[:], xct.rearrange("(q p) n -> p q n", p=128))
            id_sb = cpool.tile([128, 128], DT)
            nc.sync.dma_start(id_sb[:], ident[:])
            xgt_sb = cpool.tile([KI, B * NL], DT)
            nc.sync.dma_start(xgt_sb[0:CI, :], xgt0[:])

            # ---- stage B: E_lT[e, n] = sum_m conv_E[m, e] * A1T[m, n] ----
            elt_ps = pp.tile([128, 512], DT, tag="mm512", name="elt_ps")[:E, :NL]
            for mc in range(MCH):
                a1t_t = spool.tile([128, NL], DT, tag="a1t")
                nc.sync.dma_start(a1t_t[:], a1t[mc * 128:(mc + 1) * 128, :])
                nc.tensor.matmul(
                    elt_ps[:], e_sb[:, mc, :], a1t_t[:],
                    start=(mc == 0), stop=(mc == MCH - 1),
                )
            elt_sb = cpool.tile([E, NL], DT)
            nc.vector.tensor_copy(elt_sb[:], elt_ps[:])

            # ---- stage C: scores + softmax (rows n_c, natural layout) ----
            # expS[n, m] = exp(relu(scores) - rowmax); rows normalized by 1/rowsum
            exps = [cpool.tile([128, N], DT, tag=f"exps{h}", name=f"exps{h}") for h in range(2)]
            for h in range(2):
                rmax = spool.tile([128, 1], DT, tag="rmax")
                nmax = spool.tile([128, 1], DT, tag="nmax")
                rsum = spool.tile([128, 1], DT, tag="rsum")
                rrec = spool.tile([128, 1], DT, tag="rrec")
                for mt in range(4):
                    sc_ps = pp.tile([128, 512], DT, tag="mm512")
                    nc.tensor.matmul(
                        sc_ps[:],
                        elt_sb[:, h * 128:(h + 1) * 128],
                        et_sb[:, mt * 512:(mt + 1) * 512],
                        start=True, stop=True,
                    )
                    # relu into SBUF (ACT engine, PSUM -> SBUF)
                    nc.scalar.activation(
                        exps[h][:, mt * 512:(mt + 1) * 512], sc_ps[:],
                        mybir.ActivationFunctionType.Relu,
                    )
                nc.vector.tensor_reduce(
                    rmax[:], exps[h][:], axis=mybir.AxisListType.X,
                    op=mybir.AluOpType.max,
                )
                nc.vector.tensor_scalar_mul(nmax[:], rmax[:], -1.0)
                nc.scalar.activation(
                    exps[h][:], exps[h][:], mybir.ActivationFunctionType.Exp,
                    bias=nmax[:], accum_out=rsum[:],
                )
                nc.vector.reciprocal(rrec[:], rsum[:])
                nc.vector.tensor_scalar_mul(exps[h][:], exps[h][:], rrec[:])

            # ---- stage D: transpose S -> ST[m, n] (PE transpose, 32 blocks) ----
            st_sb = cpool.tile([128, MCH, NL], DT)
            for h in range(2):
                for mc in range(MCH):
                    tp = pp.tile([128, 128], DT, tag="tp")
                    nc.tensor.transpose(
                        tp[:], exps[h][:, mc * 128:(mc + 1) * 128], id_sb[:]
                    )
                    nc.vector.tensor_copy(
                        st_sb[:, mc, h * 128:(h + 1) * 128], tp[:]
                    )

            # ---- stage E: xg1[n, bc] = ST.T @ X  (rows n_c), -> DRAM for AG ----
            for h in range(2):
                ps1 = pp.tile([128, BC], DT, tag="mm512")
                for mc in range(MCH):
                    nc.tensor.matmul(
                        ps1[:],
                        st_sb[:, mc, h * 128:(h + 1) * 128],
                        xf_sb[:, mc, :],
                        start=(mc == 0), stop=(mc == MCH - 1),
                    )
                xg1_sb = spool.tile([128, BC], DT, tag="xg1")
                nc.vector.tensor_copy(xg1_sb[:], ps1[:])
                nc.sync.dma_start(xg1_own[h * 128:(h + 1) * 128, :], xg1_sb[:])

            # ---- stage F: AllGather xg1 -> xg1_all [N, BC] ----
            nc.gpsimd.collective_compute(
                "AllGather",
                mybir.AluOpType.bypass,
                replica_groups=[list(range(NCORE))],
                ins=[xg1_own[:]],
                outs=[xg1_all[:]],
            )

            # ---- stage G: xg1T / xg2T in [bc, n] layout ----
            # xg1T[bc, n] = sum_m X[m, bc] * ST[m, n]
            # xg2T[bc, n] = 2 * sum_m XG1[m, bc] * ST[m, n] - XcT[bc, n]
            xg1t = [spool.tile([128, NL], DT, tag=f"xg1t{q}", name=f"xg1t{q}") for q in range(4)]
            xg2t = [spool.tile([128, NL], DT, tag=f"xg2t{q}", name=f"xg2t{q}") for q in range(4)]
            for q in range(4):
                pst1 = pp.tile([128, NL], DT, tag="pst1", bufs=1)
                pst2 = pp.tile([128, NL], DT, tag="pst2", bufs=1)
                for mc in range(MCH):
                    ga_t = spool.tile([128, 128], DT, tag="ga")
                    nc.sync.dma_start(
                        ga_t[:],
                        xg1_all[mc * 128:(mc + 1) * 128, q * 128:(q + 1) * 128],
                    )
                    nc.tensor.matmul(
                        pst1[:],
                        xf_sb[:, mc, q * 128:(q + 1) * 128],
                        st_sb[:, mc, :],
                        start=(mc == 0), stop=(mc == MCH - 1),
                    )
                    nc.tensor.matmul(
                        pst2[:], ga_t[:], st_sb[:, mc, :],
                        start=(mc == 0), stop=(mc == MCH - 1),
                    )
                nc.vector.tensor_copy(xg1t[q][:], pst1[:])
                # xg2T = 2*pst2 - XcT
                nc.vector.scalar_tensor_tensor(
                    xg2t[q][:], pst2[:], 2.0, xct_sb[:, q, :],
                    op0=mybir.AluOpType.mult, op1=mybir.AluOpType.subtract,
                )

            # ---- remap into xgT[ki, b*NL+n] via SBUF->SBUF DMA ----
            for b in range(B):
                q, r = divmod(b, 4)
                nc.sync.dma_start(
                    xgt_sb[CI:2 * CI, b * NL:(b + 1) * NL],
                    xg1t[q][r * 32:(r + 1) * 32, :],
                )
                nc.sync.dma_start(
                    xgt_sb[2 * CI:3 * CI, b * NL:(b + 1) * NL],
                    xg2t[q][r * 32:(r + 1) * 32, :],
                )

            # ---- stage H: out[b, n, o] = sum_ki xgT[ki, b, n] * WT[ki, n, o] ----
            out_sb = cpool.tile([128, 8 * 512], DT)
            for t in range(8):           # 32 nodes per tile
                wt_t = wpool.tile([KI, 32 * CO], DT, tag="wt")
                nc.sync.dma_start(
                    wt_t[:], wt[:, t * 32 * CO:(t + 1) * 32 * CO]
                )
                pso = pp.tile([128, 512], DT, tag="pso")
                nc.vector.memset(pso[:], 0.0)
                for g in range(4):
                    for j in range(8):
                        n = g * 8 + j     # node within tile t
                        nn = t * 32 + n   # node within core
                        nc.tensor.matmul(
                            pso[g * 32:g * 32 + B, j * 64:(j + 1) * 64],
                            xgt_sb[:, nn::NL],
                            wt_t[:, n * CO:(n + 1) * CO],
                            start=True, stop=True,
                            tile_position=(0, g * 32),
                        )
                nc.vector.tensor_copy(out_sb[:, t * 512:(t + 1) * 512], pso[:])

            # ---- output DMA: out[b, n=32t+8g+j, o] ----
            outd_r = outd.rearrange("b (t g j) o -> b t g j o", t=8, g=4, j=8)
            for g in range(4):
                nc.sync.dma_start(
                    outd_r[:, :, g, :, :],
                    out_sb[32 * g:32 * g + B, :],
                )

    _split_matmul_waits(nc)
    return nc


def _split_matmul_waits(nc):
    """walrus encodes at most one sync-wait per TPB instruction (the EVENTS
    struct has a single wait slot); hoist extra waits onto preceding
    same-engine no-ops."""
    f = nc.m.functions[0]
    for blk in f.blocks:
        insts = blk.instructions
        out = []
        changed = False
        for inst in insts:
            si = inst.sync_info
            if (
                si is not None and si.on_wait and len(si.on_wait) > 1
            ):
                waits = list(si.on_wait)
                for k, w in enumerate(waits[:-1]):
                    nop = mybir.InstNoOp(
                        name=f"{inst.name}-wsplit-{k}",
                        engine=inst.engine,
                        sync_info=mybir.SyncInfo(on_wait=[w], on_update=[]),
                    )
                    out.append(nop)
                inst.sync_info = mybir.SyncInfo(
                    on_wait=[waits[-1]], on_update=list(si.on_update or [])
                )
                changed = True
            out.append(inst)
        if changed:
            blk.instructions = out


def kernel(A, x, conv_E, weight, b):
    A = np.asarray(A, dtype=np.float32)
    x = np.asarray(x, dtype=np.float32)
    conv_E = np.asarray(conv_E, dtype=np.float32)
    weight = np.asarray(weight, dtype=np.float32)
    b = np.asarray(b, dtype=np.float32)

    if "nc" not in _CACHE:
        _CACHE["nc"] = _build()
    nc = _CACHE["nc"]

    X = np.ascontiguousarray(x.transpose(1, 0, 2).reshape(N, BC))
    ET = np.ascontiguousarray(conv_E.T)
    IDENT = np.eye(128, dtype=np.float32)
    A1 = A[1]

    in_maps = []
    for c in range(NCORE):
        sl = slice(c * NL, (c + 1) * NL)
        a1t_c = np.ascontiguousarray(A1[sl, :].T)                       # [N, NL]
        xct_c = np.ascontiguousarray(X[sl, :].T)                        # [BC, NL]
        xgt0_c = np.ascontiguousarray(
            xct_c.reshape(B, CI, NL).transpose(1, 0, 2).reshape(CI, B * NL)
        )
        wt_c = np.ascontiguousarray(
            weight[sl].transpose(1, 2, 0, 3).reshape(KI, NL * CO)
        )
        in_maps.append({
            "a1t": a1t_c, "et": ET, "e_nat": conv_E, "xfull": X,
            "xct": xct_c, "xgt0": xgt0_c, "wt": wt_c, "ident": IDENT,
        })

    res = run_bass_kernel_spmd(nc, in_maps, core_ids=list(range(NCORE)), **_RUN_KWARGS)
    _LAST_RESULT[0] = res
    full = np.concatenate([res.results[c]["out"] for c in range(NCORE)], axis=1)
    return (full + b[None, None, :]).astype(np.float32)


# revision 16
# speedup vs baseline: 1.7093x; 1.7093x over previous
"""Trainium2 Bass kernel for nn_GCN_12979391169341 (GNN message passing).

Reference computation (N=2048 nodes, B=16, C_IN=32, C_OUT=64, E=16, K=3):
    A1 = A[1]
    conv_E_l = A1 @ conv_E
    scores = relu(conv_E_l @ conv_E.T)
    supports S = softmax(scores, axis=1)
    S_set = [I, S, 2*S@S - I]           (Chebyshev)
    x_g[b,n,k,c] = sum_m S_k[n,m] x[b,m,c]
    out[b,n,o]   = sum_{k,i} x_g[b,n,k,i] * weight[n,k,i,o] + b

Sharding: node-parallel over 8 cores (256 nodes each). Each core computes
its 256 rows of S locally (softmax rows are independent), computes its rows
of x_g1 = S@x, all-gathers x_g1 (4MB) so x_g2 = 2*S@(S@x) - x avoids ever
materializing S^2, then applies the per-node weights.

Key layout trick: S is computed in natural [n, m] layout (softmax along the
free axis), then PE-transposed to [m, n] so it can serve as the stationary
operand of every downstream matmul.
"""

import numpy as np
import sys

sys.path.insert(0, "/opt/trn_rl_repo")

import concourse.bass as bass
import concourse.mybir as mybir
import concourse.tile as tile
from concourse.bass_utils import run_bass_kernel_spmd

N = 2048      # nodes
B = 16        # batch
CI = 32       # in channels
CO = 64       # out channels
E = 16        # conv_E dim
KCH = 3       # Chebyshev order
NCORE = 8
NL = N // NCORE          # 256 local nodes
BC = B * CI              # 512
KI = KCH * CI            # 96
DT = mybir.dt.float32

_CACHE = {}
_SPLIT_WAITS = [True]
_RUN_KWARGS = {}
_LAST_RESULT = [None]


def _build():
    nc = bass.Bass(num_devices=NCORE)
    BF = mybir.dt.bfloat16

    # ---- I/O ----
    # fp32 inputs (softmax chain needs full precision)
    a1t = nc.dram_tensor("a1t", [N, NL], DT, kind="ExternalInput")      # A1[nc,:].T
    et = nc.dram_tensor("et", [E, N], DT, kind="ExternalInput")         # conv_E.T
    e_nat = nc.dram_tensor("e_nat", [N, E], DT, kind="ExternalInput")   # conv_E
    xct = nc.dram_tensor("xct", [BC, NL], DT, kind="ExternalInput")     # X[nc,:].T
    # bf16 inputs (heavy matmul operands)
    xfull = nc.dram_tensor("xfull", [N, BC], BF, kind="ExternalInput")  # X[m, b*CI+i]
    xgt0 = nc.dram_tensor("xgt0", [CI, B * NL], BF, kind="ExternalInput")  # [i, b*NL+n]
    wt = nc.dram_tensor("wt", [KI, NL * CO], BF, kind="ExternalInput")  # [ki, n*CO+o]
    ident = nc.dram_tensor("ident", [128, 128], BF, kind="ExternalInput")
    outd = nc.dram_tensor("out", [B, NL, CO], DT, kind="ExternalOutput")

    # internal DRAM for the collective (bf16 payload)
    xg1_own = nc.dram_tensor("xg1_own", [NL, BC], BF)
    xg1_all = nc.dram_tensor("xg1_all", [N, BC], BF)

    MCH = N // 128  # 16 m-chunks

    with tile.TileContext(nc) as tc:
        with (
            tc.tile_pool(name="const", bufs=1) as cpool,
            tc.tile_pool(name="stream", bufs=3) as spool,
            tc.tile_pool(name="psum", bufs=2, space="PSUM") as pp,
        ):
            # ---- resident loads (one DMA each) ----
            et_sb = cpool.tile([E, N], DT)
            nc.sync.dma_start(et_sb[:], et[:])
            e_sb = cpool.tile([128, MCH, E], DT)
            nc.sync.dma_start(e_sb[:], e_nat.rearrange("(c p) e -> p c e", p=128))
            a1t_sb = cpool.tile([128, MCH, NL], DT)
            nc.sync.dma_start(a1t_sb[:], a1t.rearrange("(c p) n -> p c n", p=128))
            xf_sb = cpool.tile([128, MCH, BC], BF)
            nc.sync.dma_start(xf_sb[:], xfull.rearrange("(c p) n -> p c n", p=128))
            xct_sb = cpool.tile([128, 4, NL], DT)
            nc.sync.dma_start(xct_sb[:], xct.rearrange("(q p) n -> p q n", p=128))
            id_sb = cpool.tile([128, 128], BF)
            nc.sync.dma_start(id_sb[:], ident[:])
            xgt_sb = cpool.tile([KI, B * NL], BF)
            nc.gpsimd.dma_start(xgt_sb[0:CI, :], xgt0[:])
            wt_sb = cpool.tile([KI, NL * CO], BF)
            nc.gpsimd.dma_start(wt_sb[:], wt[:])

            # ---- stage B: E_lT[e, n] = sum_m conv_E[m, e] * A1T[m, n] (fp32) ----
            elt_ps = pp.tile([128, 512], DT, tag="mm512", name="elt_ps")[:E, :NL]
            for mc in range(MCH):
                nc.tensor.matmul(
                    elt_ps[:], e_sb[:, mc, :], a1t_sb[:, mc, :],
                    start=(mc == 0), stop=(mc == MCH - 1),
                )
            elt_sb = cpool.tile([E, NL], DT)
            nc.vector.tensor_copy(elt_sb[:], elt_ps[:])

            # ---- stage C: scores + softmax (fp32 scores, bf16 normalized S) ----
            exps = [cpool.tile([128, N], BF, tag=f"exps{h}", name=f"exps{h}") for h in range(2)]
            for h in range(2):
                relu_sb = spool.tile([128, N], DT, tag="relu", bufs=2)
                rmax = spool.tile([128, 1], DT, tag="rmax")
                nmax = spool.tile([128, 1], DT, tag="nmax")
                rsum = spool.tile([128, 1], DT, tag="rsum")
                rrec = spool.tile([128, 1], DT, tag="rrec")
                for mt in range(4):
                    sc_ps = pp.tile([128, 512], DT, tag="mm512", name="sc_ps")
                    nc.tensor.matmul(
                        sc_ps[:],
                        elt_sb[:, h * 128:(h + 1) * 128],
                        et_sb[:, mt * 512:(mt + 1) * 512],
                        start=True, stop=True,
                    )
                    nc.scalar.activation(
                        relu_sb[:, mt * 512:(mt + 1) * 512], sc_ps[:],
                        mybir.ActivationFunctionType.Relu,
                    )
                nc.vector.tensor_reduce(
                    rmax[:], relu_sb[:], axis=mybir.AxisListType.X,
                    op=mybir.AluOpType.max,
                )
                nc.vector.tensor_scalar_mul(nmax[:], rmax[:], -1.0)
                nc.scalar.activation(
                    exps[h][:], relu_sb[:], mybir.ActivationFunctionType.Exp,
                    bias=nmax[:], accum_out=rsum[:],
                )
                nc.vector.reciprocal(rrec[:], rsum[:])
                nc.vector.tensor_scalar_mul(exps[h][:], exps[h][:], rrec[:])

            # ---- stage D: transpose S -> ST[m, n] (PE transpose, bf16) ----
            st_sb = cpool.tile([128, MCH, NL], BF)
            for h in range(2):
                for mc in range(MCH):
                    tp = pp.tile([128, 128], BF, tag="tp")
                    nc.tensor.transpose(
                        tp[:], exps[h][:, mc * 128:(mc + 1) * 128], id_sb[:]
                    )
                    nc.vector.tensor_copy(
                        st_sb[:, mc, h * 128:(h + 1) * 128], tp[:]
                    )

            # ---- stage E: xg1[n, bc] = ST.T @ X  -> DRAM, AllGather ----
            for h in range(2):
                ps1 = pp.tile([128, BC], DT, tag="mm512", name="ps1")
                for mc in range(MCH):
                    nc.tensor.matmul(
                        ps1[:],
                        st_sb[:, mc, h * 128:(h + 1) * 128],
                        xf_sb[:, mc, :],
                        start=(mc == 0), stop=(mc == MCH - 1),
                    )
                xg1_sb = spool.tile([128, BC], BF, tag="xg1")
                nc.vector.tensor_copy(xg1_sb[:], ps1[:])
                nc.sync.dma_start(xg1_own[h * 128:(h + 1) * 128, :], xg1_sb[:])

            # ---- stage F: AllGather xg1 -> xg1_all [N, BC] ----
            nc.gpsimd.collective_compute(
                "AllGather",
                mybir.AluOpType.bypass,
                replica_groups=[list(range(NCORE))],
                ins=[xg1_own[:]],
                outs=[xg1_all[:]],
            )

            # ---- stage G1: xg1T[bc, n] = X.T-chunks vs ST (independent of AG) ----
            xg1t_all = cpool.tile([128, 4, NL], BF)
            xg2t_all = cpool.tile([128, 4, NL], BF)
            for q in range(4):
                pst1 = pp.tile([128, NL], DT, tag="pst1", bufs=1)
                for mc in range(MCH):
                    nc.tensor.matmul(
                        pst1[:],
                        xf_sb[:, mc, q * 128:(q + 1) * 128],
                        st_sb[:, mc, :],
                        start=(mc == 0), stop=(mc == MCH - 1),
                    )
                nc.vector.tensor_copy(xg1t_all[:, q, :], pst1[:])

            # ---- stage G2: xg2T[bc, n] = 2 * XG1.T-chunks vs ST - XcT ----
            ga_sb = cpool.tile([128, MCH, BC], BF)
            nc.sync.dma_start(ga_sb[:], xg1_all.rearrange("(c p) n -> p c n", p=128))
            for q in range(4):
                pst2 = pp.tile([128, NL], DT, tag="pst2", bufs=1)
                for mc in range(MCH):
                    nc.tensor.matmul(
                        pst2[:],
                        ga_sb[:, mc, q * 128:(q + 1) * 128],
                        st_sb[:, mc, :],
                        start=(mc == 0), stop=(mc == MCH - 1),
                    )
                nc.vector.scalar_tensor_tensor(
                    xg2t_all[:, q, :], pst2[:], 2.0, xct_sb[:, q, :],
                    op0=mybir.AluOpType.mult, op1=mybir.AluOpType.subtract,
                )

            # ---- remap into xgT[ki, b*NL+n]: one DMA per (k, r) ----
            # b = 4*q + r; src partition r*32+i, free (q, n);
            # dst partition k*CI+i, free b*NL+n = 1024*q + 256*r + n
            for k, srct in ((1, xg1t_all), (2, xg2t_all)):
                for r in range(4):
                    nc.gpsimd.dma_start(
                        xgt_sb[k * CI:(k + 1) * CI, :].rearrange(
                            "i (q r n) -> i q r n", q=4, r=4, n=NL
                        )[:, :, r, :],
                        srct[r * 32:(r + 1) * 32, :, :],
                    )

            # ---- stage H: out[b, n, o] = sum_ki xgT[ki, b, n] * WT[ki, n, o] ----
            out_sb = cpool.tile([128, 8 * 512], DT)
            for t in range(8):           # 32 nodes per tile
                pso = pp.tile([128, 512], DT, tag="pso")
                nc.vector.memset(pso[:], 0.0)
                for g in range(4):
                    for j in range(8):
                        n = g * 8 + j     # node within tile t
                        nn = t * 32 + n   # node within core
                        nc.tensor.matmul(
                            pso[g * 32:g * 32 + B, j * 64:(j + 1) * 64],
                            xgt_sb[:, nn::NL],
                            wt_sb[:, nn * CO:(nn + 1) * CO],
                            start=True, stop=True,
                            tile_position=(0, g * 32),
                        )
                nc.vector.tensor_copy(out_sb[:, t * 512:(t + 1) * 512], pso[:])

            # ---- output DMA: out[b, n=32t+8g+j, o] ----
            outd_r = outd.rearrange("b (t g j) o -> b t g j o", t=8, g=4, j=8)
            for g in range(4):
                nc.sync.dma_start(
                    outd_r[:, :, g, :, :],
                    out_sb[32 * g:32 * g + B, :],
                )

    if _SPLIT_WAITS[0]:
        _split_matmul_waits(nc)
    return nc


def _split_matmul_waits(nc):
    """walrus encodes at most one sync-wait per TPB instruction (the EVENTS
    struct has a single wait slot); hoist extra waits onto preceding
    same-engine no-ops."""
    f = nc.m.functions[0]
    for blk in f.blocks:
        insts = blk.instructions
        out = []
        changed = False
        for inst in insts:
            si = inst.sync_info
            if (
                si is not None and si.on_wait and len(si.on_wait) > 1
            ):
                waits = list(si.on_wait)
                for k, w in enumerate(waits[:-1]):
                    nop = mybir.InstNoOp(
                        name=f"{inst.name}-wsplit-{k}",
                        engine=inst.engine,
                        sync_info=mybir.SyncInfo(on_wait=[w], on_update=[]),
                    )
                    out.append(nop)
                inst.sync_info = mybir.SyncInfo(
                    on_wait=[waits[-1]], on_update=list(si.on_update or [])
                )
                changed = True
            out.append(inst)
        if changed:
            blk.instructions = out


def kernel(A, x, conv_E, weight, b):
    A = np.asarray(A, dtype=np.float32)
    x = np.asarray(x, dtype=np.float32)
    conv_E = np.asarray(conv_E, dtype=np.float32)
    weight = np.asarray(weight, dtype=np.float32)
    b = np.asarray(b, dtype=np.float32)

    if "nc" not in _CACHE:
        _CACHE["nc"] = _build()
    nc = _CACHE["nc"]

    import ml_dtypes
    BF = ml_dtypes.bfloat16

    X = np.ascontiguousarray(x.transpose(1, 0, 2).reshape(N, BC))
    X_bf = X.astype(BF)
    ET = np.ascontiguousarray(conv_E.T)
    IDENT = np.eye(128, dtype=BF)
    A1 = A[1]

    in_maps = []
    for c in range(NCORE):
        sl = slice(c * NL, (c + 1) * NL)
        a1t_c = np.ascontiguousarray(A1[sl, :].T)                       # [N, NL]
        xct_c = np.ascontiguousarray(X[sl, :].T)                        # [BC, NL]
        xgt0_c = np.ascontiguousarray(
            xct_c.reshape(B, CI, NL).transpose(1, 0, 2).reshape(CI, B * NL)
        ).astype(BF)
        wt_c = np.ascontiguousarray(
            weight[sl].transpose(1, 2, 0, 3).reshape(KI, NL * CO)
        ).astype(BF)
        in_maps.append({
            "a1t": a1t_c, "et": ET, "e_nat": conv_E, "xfull": X_bf,
            "xct": xct_c, "xgt0": xgt0_c, "wt": wt_c, "ident": IDENT,
        })

    res = run_bass_kernel_spmd(nc, in_maps, core_ids=list(range(NCORE)), **_RUN_KWARGS)
    _LAST_RESULT[0] = res
    full = np.concatenate([res.results[c]["out"] for c in range(NCORE)], axis=1)
    return (full + b[None, None, :]).astype(np.float32)


# revision 18
# speedup vs baseline: 1.9962x; 1.1678x over previous
"""Trainium2 Bass kernel for nn_GCN_12979391169341 (GNN message passing).

Reference computation (N=2048 nodes, B=16, C_IN=32, C_OUT=64, E=16, K=3):
    A1 = A[1]
    conv_E_l = A1 @ conv_E
    scores = relu(conv_E_l @ conv_E.T)
    supports S = softmax(scores, axis=1)
    S_set = [I, S, 2*S@S - I]           (Chebyshev)
    x_g[b,n,k,c] = sum_m S_k[n,m] x[b,m,c]
    out[b,n,o]   = sum_{k,i} x_g[b,n,k,i] * weight[n,k,i,o] + b

Sharding: node-parallel over 8 cores (256 nodes each). Each core computes
its 256 rows of S locally (softmax rows are independent), computes its rows
of x_g1 = S@x, all-gathers x_g1 (4MB) so x_g2 = 2*S@(S@x) - x avoids ever
materializing S^2, then applies the per-node weights.

Key layout trick: S is computed in natural [n, m] layout (softmax along the
free axis), then PE-transposed to [m, n] so it can serve as the stationary
operand of every downstream matmul.
"""

import numpy as np
import sys

sys.path.insert(0, "/opt/trn_rl_repo")

import concourse.bass as bass
import concourse.mybir as mybir
import concourse.tile as tile
from concourse.bass_utils import run_bass_kernel_spmd

N = 2048      # nodes
B = 16        # batch
CI = 32       # in channels
CO = 64       # out channels
E = 16        # conv_E dim
KCH = 3       # Chebyshev order
NCORE = 8
NL = N // NCORE          # 256 local nodes
BC = B * CI              # 512
KI = KCH * CI            # 96
DT = mybir.dt.float32

_CACHE = {}
_SPLIT_WAITS = [True]
_SHARED_AG = [True]
_RUN_KWARGS = {}
_LAST_RESULT = [None]


def _build():
    nc = bass.Bass(num_devices=NCORE)
    BF = mybir.dt.bfloat16

    # ---- I/O ----
    # fp32 inputs (softmax chain needs full precision)
    a1t = nc.dram_tensor("a1t", [N, NL], DT, kind="ExternalInput")      # A1[nc,:].T
    et = nc.dram_tensor("et", [E, N], DT, kind="ExternalInput")         # conv_E.T
    e_nat = nc.dram_tensor("e_nat", [N, E], DT, kind="ExternalInput")   # conv_E
    xct = nc.dram_tensor("xct", [BC, NL], DT, kind="ExternalInput")     # X[nc,:].T
    # bf16 inputs (heavy matmul operands)
    xfull = nc.dram_tensor("xfull", [N, BC], BF, kind="ExternalInput")  # X[m, b*CI+i]
    xgt0 = nc.dram_tensor("xgt0", [CI, B * NL], BF, kind="ExternalInput")  # [i, b*NL+n]
    wt = nc.dram_tensor("wt", [KI, NL * CO], BF, kind="ExternalInput")  # [ki, n*CO+o]
    ident = nc.dram_tensor("ident", [128, 128], BF, kind="ExternalInput")
    outd = nc.dram_tensor("out", [B, NL, CO], DT, kind="ExternalOutput")

    # internal DRAM for the collective (bf16 payload)
    xg1_own = nc.dram_tensor("xg1_own", [NL, BC], BF)
    xg1_all = nc.dram_tensor(
        "xg1_all", [N, BC], BF,
        addr_space="Shared" if _SHARED_AG[0] else "Local",
    )

    MCH = N // 128  # 16 m-chunks

    with tile.TileContext(nc) as tc:
        with (
            tc.tile_pool(name="const", bufs=1) as cpool,
            tc.tile_pool(name="stream", bufs=3) as spool,
            tc.tile_pool(name="psum", bufs=2, space="PSUM") as pp,
        ):
            # ---- resident loads (one DMA each) ----
            engs = [nc.sync, nc.scalar, nc.gpsimd, nc.sync]
            et_sb = cpool.tile([E, N], DT)
            nc.sync.dma_start(et_sb[:], et[:])
            e_sb = cpool.tile([128, MCH, E], DT)
            nc.scalar.dma_start(e_sb[:], e_nat.rearrange("(c p) e -> p c e", p=128))
            a1t_sb = cpool.tile([128, MCH, NL], DT)
            a1t_r = a1t.rearrange("(c p) n -> p c n", p=128)
            for i in range(4):
                engs[i].dma_start(
                    a1t_sb[:, 4 * i:4 * (i + 1), :], a1t_r[:, 4 * i:4 * (i + 1), :]
                )
            xf_sb = cpool.tile([128, MCH, BC], BF)
            xf_r = xfull.rearrange("(c p) n -> p c n", p=128)
            for i in range(4):
                engs[i].dma_start(
                    xf_sb[:, 4 * i:4 * (i + 1), :], xf_r[:, 4 * i:4 * (i + 1), :]
                )
            xct_sb = cpool.tile([128, 4, NL], DT)
            nc.scalar.dma_start(xct_sb[:], xct.rearrange("(q p) n -> p q n", p=128))
            id_sb = cpool.tile([128, 128], BF)
            nc.sync.dma_start(id_sb[:], ident[:])
            xgt_sb = cpool.tile([KI, B * NL], BF)
            nc.gpsimd.dma_start(xgt_sb[0:CI, :], xgt0[:])
            wt_sb = cpool.tile([KI, NL * CO], BF)
            for i in range(4):
                engs[i].dma_start(
                    wt_sb[:, 4096 * i:4096 * (i + 1)], wt[:, 4096 * i:4096 * (i + 1)]
                )

            # ---- stage B: E_lT[e, n] = sum_m conv_E[m, e] * A1T[m, n] (fp32) ----
            elt_ps = pp.tile([128, 512], DT, tag="mm512", name="elt_ps")[:E, :NL]
            for mc in range(MCH):
                nc.tensor.matmul(
                    elt_ps[:], e_sb[:, mc, :], a1t_sb[:, mc, :],
                    start=(mc == 0), stop=(mc == MCH - 1),
                )
            elt_sb = cpool.tile([E, NL], DT)
            nc.vector.tensor_copy(elt_sb[:], elt_ps[:])

            # ---- stage C: scores + softmax (fp32 scores, bf16 normalized S) ----
            exps = [cpool.tile([128, N], BF, tag=f"exps{h}", name=f"exps{h}") for h in range(2)]
            for h in range(2):
                relu_sb = spool.tile([128, N], DT, tag="relu", bufs=2)
                rmax = spool.tile([128, 1], DT, tag="rmax")
                nmax = spool.tile([128, 1], DT, tag="nmax")
                rsum = spool.tile([128, 1], DT, tag="rsum")
                rrec = spool.tile([128, 1], DT, tag="rrec")
                for mt in range(4):
                    sc_ps = pp.tile([128, 512], DT, tag="mm512", name="sc_ps")
                    nc.tensor.matmul(
                        sc_ps[:],
                        elt_sb[:, h * 128:(h + 1) * 128],
                        et_sb[:, mt * 512:(mt + 1) * 512],
                        start=True, stop=True,
                    )
                    nc.scalar.activation(
                        relu_sb[:, mt * 512:(mt + 1) * 512], sc_ps[:],
                        mybir.ActivationFunctionType.Relu,
                    )
                nc.vector.tensor_reduce(
                    rmax[:], relu_sb[:], axis=mybir.AxisListType.X,
                    op=mybir.AluOpType.max,
                )
                nc.vector.tensor_scalar_mul(nmax[:], rmax[:], -1.0)
                nc.scalar.activation(
                    exps[h][:], relu_sb[:], mybir.ActivationFunctionType.Exp,
                    bias=nmax[:], accum_out=rsum[:],
                )
                nc.vector.reciprocal(rrec[:], rsum[:])
                nc.vector.tensor_scalar_mul(exps[h][:], exps[h][:], rrec[:])

            # ---- stage D: transpose S -> ST[m, n] (PE transpose, bf16) ----
            st_sb = cpool.tile([128, MCH, NL], BF)
            for h in range(2):
                for mc in range(MCH):
                    tp = pp.tile([128, 128], BF, tag="tp")
                    nc.tensor.transpose(
                        tp[:], exps[h][:, mc * 128:(mc + 1) * 128], id_sb[:]
                    )
                    nc.vector.tensor_copy(
                        st_sb[:, mc, h * 128:(h + 1) * 128], tp[:]
                    )

            # ---- stage E: xg1[n, bc] = ST.T @ X  -> DRAM, AllGather ----
            for h in range(2):
                ps1 = pp.tile([128, BC], DT, tag="mm512", name="ps1")
                for mc in range(MCH):
                    nc.tensor.matmul(
                        ps1[:],
                        st_sb[:, mc, h * 128:(h + 1) * 128],
                        xf_sb[:, mc, :],
                        start=(mc == 0), stop=(mc == MCH - 1),
                    )
                xg1_sb = spool.tile([128, BC], BF, tag="xg1")
                nc.vector.tensor_copy(xg1_sb[:], ps1[:])
                nc.sync.dma_start(xg1_own[h * 128:(h + 1) * 128, :], xg1_sb[:])

            # ---- stage F: AllGather xg1 -> xg1_all [N, BC] ----
            nc.gpsimd.collective_compute(
                "AllGather",
                mybir.AluOpType.bypass,
                replica_groups=[list(range(NCORE))],
                ins=[xg1_own[:]],
                outs=[xg1_all[:]],
            )

            # ---- stage G1: xg1T[bc, n] = X.T-chunks vs ST (independent of AG) ----
            xg1t_all = cpool.tile([128, 4, NL], BF)
            xg2t_all = cpool.tile([128, 4, NL], BF)
            for q in range(4):
                pst1 = pp.tile([128, NL], DT, tag="pst1", bufs=1)
                for mc in range(MCH):
                    nc.tensor.matmul(
                        pst1[:],
                        xf_sb[:, mc, q * 128:(q + 1) * 128],
                        st_sb[:, mc, :],
                        start=(mc == 0), stop=(mc == MCH - 1),
                    )
                nc.vector.tensor_copy(xg1t_all[:, q, :], pst1[:])

            # ---- stage G2: xg2T[bc, n] = 2 * XG1.T-chunks vs ST - XcT ----
            ga_sb = cpool.tile([128, MCH, BC], BF)
            ga_r = xg1_all.rearrange("(c p) n -> p c n", p=128)
            for i in range(4):
                engs[i].dma_start(
                    ga_sb[:, 4 * i:4 * (i + 1), :], ga_r[:, 4 * i:4 * (i + 1), :]
                )
            for q in range(4):
                pst2 = pp.tile([128, NL], DT, tag="pst2", bufs=1)
                for mc in range(MCH):
                    nc.tensor.matmul(
                        pst2[:],
                        ga_sb[:, mc, q * 128:(q + 1) * 128],
                        st_sb[:, mc, :],
                        start=(mc == 0), stop=(mc == MCH - 1),
                    )
                nc.vector.scalar_tensor_tensor(
                    xg2t_all[:, q, :], pst2[:], 2.0, xct_sb[:, q, :],
                    op0=mybir.AluOpType.mult, op1=mybir.AluOpType.subtract,
                )

            # ---- remap into xgT[ki, b*NL+n]: one DMA per (k, r) ----
            # b = 4*q + r; src partition r*32+i, free (q, n);
            # dst partition k*CI+i, free b*NL+n = 1024*q + 256*r + n
            for k, srct in ((1, xg1t_all), (2, xg2t_all)):
                for r in range(4):
                    nc.gpsimd.dma_start(
                        xgt_sb[k * CI:(k + 1) * CI, :].rearrange(
                            "i (q r n) -> i q r n", q=4, r=4, n=NL
                        )[:, :, r, :],
                        srct[r * 32:(r + 1) * 32, :, :],
                    )

            # ---- stage H: out[b, n, o] = sum_ki xgT[ki, b, n] * WT[ki, n, o] ----
            out_sb = cpool.tile([128, 8 * 512], DT)
            for t in range(8):           # 32 nodes per tile
                pso = pp.tile([128, 512], DT, tag="pso")
                nc.vector.memset(pso[:], 0.0)
                for g in range(4):
                    for j in range(8):
                        n = g * 8 + j     # node within tile t
                        nn = t * 32 + n   # node within core
                        nc.tensor.matmul(
                            pso[g * 32:g * 32 + B, j * 64:(j + 1) * 64],
                            xgt_sb[:, nn::NL],
                            wt_sb[:, nn * CO:(nn + 1) * CO],
                            start=True, stop=True,
                            tile_position=(0, g * 32),
                        )
                nc.vector.tensor_copy(out_sb[:, t * 512:(t + 1) * 512], pso[:])

            # ---- output DMA: out[b, n=32t+8g+j, o] ----
            outd_r = outd.rearrange("b (t g j) o -> b t g j o", t=8, g=4, j=8)
            for g in range(4):
                nc.sync.dma_start(
                    outd_r[:, :, g, :, :],
                    out_sb[32 * g:32 * g + B, :],
                )

    if _SPLIT_WAITS[0]:
        _split_matmul_waits(nc)
    return nc


def _split_matmul_waits(nc):
    """walrus encodes at most one sync-wait per TPB instruction (the EVENTS
    struct has a single wait slot); hoist extra waits onto preceding
    same-engine no-ops."""
    f = nc.m.functions[0]
    for blk in f.blocks:
        insts = blk.instructions
        out = []
        changed = False
        for inst in insts:
            si = inst.sync_info
            if (
                si is not None and si.on_wait and len(si.on_wait) > 1
            ):
                waits = list(si.on_wait)
                for k, w in enumerate(waits[:-1]):
                    nop = mybir.InstNoOp(
                        name=f"{inst.name}-wsplit-{k}",
                        engine=inst.engine,
                        sync_info=mybir.SyncInfo(on_wait=[w], on_update=[]),
                    )
                    out.append(nop)
                inst.sync_info = mybir.SyncInfo(
                    on_wait=[waits[-1]], on_update=list(si.on_update or [])
                )
                changed = True
            out.append(inst)
        if changed:
            blk.instructions = out


def kernel(A, x, conv_E, weight, b):
    A = np.asarray(A, dtype=np.float32)
    x = np.asarray(x, dtype=np.float32)
    conv_E = np.asarray(conv_E, dtype=np.float32)
    weight = np.asarray(weight, dtype=np.float32)
    b = np.asarray(b, dtype=np.float32)

    if "nc" not in _CACHE:
        _CACHE["nc"] = _build()
    nc = _CACHE["nc"]

    import ml_dtypes
    BF = ml_dtypes.bfloat16

    X = np.ascontiguousarray(x.transpose(1, 0, 2).reshape(N, BC))
    X_bf = X.astype(BF)
    ET = np.ascontiguousarray(conv_E.T)
    IDENT = np.eye(128, dtype=BF)
    A1 = A[1]

    in_maps = []
    for c in range(NCORE):
        sl = slice(c * NL, (c + 1) * NL)
        a1t_c = np.ascontiguousarray(A1[sl, :].T)                       # [N, NL]
        xct_c = np.ascontiguousarray(X[sl, :].T)                        # [BC, NL]
        xgt0_c = np.ascontiguousarray(
            xct_c.reshape(B, CI, NL).transpose(1, 0, 2).reshape(CI, B * NL)
        ).astype(BF)
        wt_c = np.ascontiguousarray(
            weight[sl].transpose(1, 2, 0, 3).reshape(KI, NL * CO)
        ).astype(BF)
        in_maps.append({
            "a1t": a1t_c, "et": ET, "e_nat": conv_E, "xfull": X_bf,
            "xct": xct_c, "xgt0": xgt0_c, "wt": wt_c, "ident": IDENT,
        })

    res = run_bass_kernel_spmd(nc, in_maps, core_ids=list(range(NCORE)), **_RUN_KWARGS)
    _LAST_RESULT[0] = res
    full = np.concatenate([res.results[c]["out"] for c in range(NCORE)], axis=1)
    return (full + b[None, None, :]).astype(np.float32)
